# revision 6
# baseline (speedup 1.0000x reference)
"""ConvLSTM (pixel-wise, 1x1 convs) Trainium2 Bass kernel.

Math (after exact algebraic folding):
  per pixel, per t:  g1 = W1x @ x_t + W1h @ h1 + b1   (W1x = Wih1 @ (W_red * denorm_scale))
                     i,f,g,o = split(g1); c1 = sig(f)*c1 + sig(i)*tanh(g); h1 = sig(o)*tanh(c1)
                     g2 = W21 @ h1 + W22 @ h2 + b2    (W21 = Wih2 @ Wc1)
                     c2,h2 analogous
  out = (W_head @ Wc2) @ h2_final + const

Sharding: batch b -> core b (8 cores, no collectives).

Per-core layout:
  S1 [92, CHUNK]   rows 0:64 = h1, rows 64:92 = x(t)    (matmul rhs, K=92)
  S2 [128, CHUNK]  rows 0:64 = h1 (dup), 64:128 = h2    (matmul rhs, K=128)
  c1/c2 [128, HALF] : A-half pixels on partitions 0:64, B-half on 64:128
  gate planes [128, FD] in PSUM: per-gate, A-half rows 0:64 / B-half rows 64:128
  -> every ACT/DVE pointwise op runs with all 128 partitions busy.

x is converted to bf16 host-side and DMA'd directly into S1's x rows each
timestep (no staging copy).  All pointwise traffic is bf16 on VectorE
(2x TT mode, 4x copy mode); GPSIMD is not used for compute.
"""

import numpy as np

import concourse.bass as bass
import concourse.tile as tile
from concourse import bacc, mybir
from concourse.bass_utils import run_bass_kernel_spmd

F32 = mybir.dt.float32
BF16 = mybir.dt.bfloat16
AF = mybir.ActivationFunctionType

T, CIN, HID = 8, 28, 64
H = W = 128
HW = H * W            # pixels per core (one batch element)
NCORES = 8
K1, K2 = HID + CIN, 2 * HID   # S1 rows 0:64 = h1, 64:92 = x; S2 rows 0:64 = h1, 64:128 = h2

import os
CFG = dict(
    chunk=8192,        # pixels resident per chunk
    fd=2048,           # PSUM gate-tile free dim (pixels per half per block)
    nt=512,            # matmul moving tile (one PSUM bank of fp32)
    planes_bufs=2,
)
for _k in list(CFG):
    _v = os.environ.get(f"KCFG_{_k.upper()}")
    if _v is not None:
        CFG[_k] = int(_v) if _v.isdigit() else _v


def _fold_weights(inputs):
    """Host-side exact algebraic folding (all fp32 numpy)."""
    f = np.float32
    W_red = inputs["W_red"].astype(f)
    b_red = inputs["b_red"].astype(f)
    # de-normalization of channels 11 (u) and 12 (v), folded into W_red
    a = np.ones(CIN, f); a[11] = f(0.15); a[12] = f(0.12)
    d = np.zeros(CIN, f); d[11] = f(0.02); d[12] = f(-0.01)
    W_red_eff = W_red * a[None, :]
    b_red_eff = b_red + W_red @ d

    W1x = inputs["Wih1"].astype(f) @ W_red_eff          # [256, 28]
    W1h = inputs["Whh1"].astype(f)                      # [256, 64]
    b1 = (inputs["bih1"] + inputs["bhh1"]).astype(f) + inputs["Wih1"].astype(f) @ b_red_eff
    W21 = inputs["Wih2"].astype(f) @ inputs["Wc1"].astype(f)   # [256, 64]
    W22 = inputs["Whh2"].astype(f)                      # [256, 64]
    b2 = (inputs["bih2"] + inputs["bhh2"]).astype(f) + inputs["Wih2"].astype(f) @ inputs["bc1"].astype(f)
    whead = (inputs["W_head"].astype(f) @ inputs["Wc2"].astype(f))[0]     # [64]
    bhead = float((inputs["W_head"].astype(f) @ inputs["bc2"].astype(f) + inputs["b_head"].astype(f)).reshape(()))

    w1 = np.ascontiguousarray(np.concatenate([W1h, W1x], axis=1).T)  # [92, 256]: h1 rows then x rows
    w2 = np.ascontiguousarray(np.concatenate([W21, W22], axis=1).T)  # [128, 256]
    # per-gate bias vectors duplicated across the two half-planes -> [128, 4]
    bdup = lambda b: np.stack([np.concatenate([b[64 * q:64 * q + 64]] * 2) for q in range(4)], axis=1)
    wh = np.zeros((128, 1), f); wh[64:, 0] = whead
    return dict(w1=w1, w2=w2, b1=np.ascontiguousarray(bdup(b1)),
                b2=np.ascontiguousarray(bdup(b2)), wh=wh,
                bh=np.full((128, 1), bhead, f))


def build(nc):
    chunk = CFG["chunk"]; fd = CFG["fd"]; nt = CFG["nt"]
    nchunk = HW // chunk
    half = chunk // 2
    nblk = half // fd
    nsub = fd // nt

    x_d = nc.dram_tensor("xt", [T, CIN, HW], BF16, kind="ExternalInput").ap()
    w1_d = nc.dram_tensor("w1", [K1, 256], F32, kind="ExternalInput").ap()
    w2_d = nc.dram_tensor("w2", [K2, 256], F32, kind="ExternalInput").ap()
    wh_d = nc.dram_tensor("wh", [128, 1], F32, kind="ExternalInput").ap()
    b1_d = nc.dram_tensor("b1", [128, 4], F32, kind="ExternalInput").ap()
    b2_d = nc.dram_tensor("b2", [128, 4], F32, kind="ExternalInput").ap()
    bh_d = nc.dram_tensor("bh", [128, 1], F32, kind="ExternalInput").ap()
    # out[i, j] = pixel j*128 + i of this core's [H, W] map (host transposes)
    out_d = nc.dram_tensor("out", [128, HW // 128], F32, kind="ExternalOutput").ap()

    with tile.TileContext(nc) as tc:
        with (
            tc.tile_pool(name="const", bufs=1) as const,
            tc.tile_pool(name="state", bufs=1) as state,
            tc.tile_pool(name="planes", bufs=CFG["planes_bufs"]) as planes,
            tc.tile_pool(name="outp", bufs=1) as outp,
            tc.tile_pool(name="psum", bufs=1, space=bass.MemorySpace.PSUM) as psum,
        ):
            # Stage weights via fp32 tiles + one convert copy each, so every
            # matmul waits on a single compute producer (the fused LDWEIGHTS
            # has very few sync-wait slots; direct multi-queue DMA deps
            # overflow it -> walrus "Too many sync wait commands").
            w1f = const.tile([K1, 256], F32, tag="w1f")
            w2f = const.tile([K2, 256], F32, tag="w2f")
            whf = const.tile([128, 1], F32, tag="whf")
            nc.sync.dma_start(w1f[:], w1_d)
            nc.sync.dma_start(w2f[:], w2_d)
            nc.sync.dma_start(whf[:], wh_d)
            w1_sb = const.tile([K1, 256], BF16, tag="w1")
            w2_sb = const.tile([K2, 256], BF16, tag="w2")
            wh_sb = const.tile([128, 1], BF16, tag="wh")
            nc.vector.tensor_copy(w1_sb[:], w1f[:])
            nc.vector.tensor_copy(w2_sb[:], w2f[:])
            nc.vector.tensor_copy(wh_sb[:], whf[:])
            b1_sb = const.tile([128, 4], F32, tag="b1")
            b2_sb = const.tile([128, 4], F32, tag="b2")
            bh_sb = const.tile([128, 1], F32, tag="bh")
            nc.sync.dma_start(b1_sb[:], b1_d)
            nc.sync.dma_start(b2_sb[:], b2_d)
            nc.sync.dma_start(bh_sb[:], bh_d)

            out_sb = outp.tile([128, HW // 128], F32, tag="osb")

            for ci in range(nchunk):
                px0 = ci * chunk
                S1 = state.tile([K1, chunk], BF16, tag="S1")
                S2 = state.tile([K2, chunk], BF16, tag="S2")
                c1 = state.tile([128, half], BF16, tag="c1")
                c2 = state.tile([128, half], BF16, tag="c2")

                for t in range(T):
                    # x(t) straight into S1's x rows (host pre-converted bf16).
                    # WAR on last t's L1 matmuls clears early (L1 runs first),
                    # so this overlaps the previous step's L2 phase.
                    nc.sync.dma_start(S1[HID:K1, :], x_d[t][:, px0:px0 + chunk])

                    for lst in (0, 1):
                        if lst == 0:
                            w_sb, b_sb, SS = w1_sb, b1_sb, S1
                            ks = slice(0, K1) if t > 0 else slice(HID, K1)
                            cc = c1
                        else:
                            w_sb, b_sb, SS = w2_sb, b2_sb, S2
                            ks = slice(0, K2) if t > 0 else slice(0, HID)
                            cc = c2

                        # full-half gate planes; ACT fills fd-wide blocks from
                        # ping-pong PSUM tiles (2 x [128, fd] f32 = all 8 banks)
                        si = planes.tile([128, half], BF16, tag="si")
                        sf = planes.tile([128, half], BF16, tag="sf")
                        tg = planes.tile([128, half], BF16, tag="tg")
                        so = planes.tile([128, half], BF16, tag="so")
                        gates = [(si, AF.Sigmoid, 0), (sf, AF.Sigmoid, 1),
                                 (tg, AF.Tanh, 2), (so, AF.Sigmoid, 3)]
                        pp = 0
                        for hb in range(nblk):
                            a0 = hb * fd            # A-half cols in S1/S2
                            b0 = half + hb * fd     # B-half cols
                            blk = slice(hb * fd, (hb + 1) * fd)
                            for (pl, fn, q) in gates:
                                if t == 0 and q == 1:
                                    continue        # f-gate unused at t=0
                                P = psum.tile([128, fd], F32, tag=f"P{pp % 2}",
                                              name=f"P{pp % 2}")
                                pp += 1
                                for s in range(nsub):
                                    for (cb, po) in ((a0, 0), (b0, 64)):
                                        nc.tensor.matmul(
                                            P[po:po + 64, s * nt:(s + 1) * nt],
                                            w_sb[ks, q * 64:(q + 1) * 64],
                                            SS[ks, cb + s * nt:cb + (s + 1) * nt],
                                        )
                                nc.scalar.activation(pl[:, blk], P[:], fn,
                                                     bias=b_sb[:, q:q + 1])
                            if t > 0:
                                t1 = planes.tile([128, fd], BF16, tag="t1")
                                t2 = planes.tile([128, fd], BF16, tag="t2")
                                nc.vector.tensor_mul(t1[:], sf[:, blk], cc[:, blk])
                                nc.vector.tensor_mul(t2[:], si[:, blk], tg[:, blk])
                                nc.vector.tensor_add(cc[:, blk], t1[:], t2[:])
                            else:
                                nc.vector.tensor_mul(cc[:, blk], si[:, blk], tg[:, blk])
                        # post-chain per block so the next layer's matmuls can
                        # start as soon as block 0's h-rows land
                        for hb in range(nblk):
                            a0 = hb * fd
                            b0 = half + hb * fd
                            blk = slice(hb * fd, (hb + 1) * fd)
                            tch = planes.tile([128, fd], BF16, tag="tc")
                            nc.scalar.activation(tch[:], cc[:, blk], AF.Tanh)
                            if lst == 0:
                                hp = planes.tile([128, fd], BF16, tag="hp")
                                nc.vector.tensor_mul(hp[:], so[:, blk], tch[:])
                                # S2's h1 rows feed this step's L2 matmuls: first.
                                nc.vector.tensor_copy(S2[0:HID, a0:a0 + fd], hp[0:64, :])
                                nc.vector.tensor_copy(S2[0:HID, b0:b0 + fd], hp[64:128, :])
                                if t < T - 1:
                                    # S1's h1 rows are only needed at t+1.
                                    nc.vector.tensor_copy(S1[0:HID, a0:a0 + fd], hp[0:64, :])
                                    nc.vector.tensor_copy(S1[0:HID, b0:b0 + fd], hp[64:128, :])
                            else:
                                nc.vector.tensor_mul(S2[HID:K2, a0:a0 + fd], so[0:64, blk], tch[0:64, :])
                                nc.vector.tensor_mul(S2[HID:K2, b0:b0 + fd], so[64:128, blk], tch[64:128, :])

                # head: out[pix] = whead @ h2[pix] + bh, pixels as matmul M-dim
                ncols = chunk // 128
                ph = psum.tile([128, ncols], F32, tag="P0", name="ph")
                for j in range(ncols):
                    nc.tensor.matmul(
                        ph[:, j:j + 1],
                        S2[HID:K2, j * 128:(j + 1) * 128],
                        wh_sb[64:128, 0:1],
                    )
                nc.vector.tensor_scalar_add(
                    out_sb[:, ci * ncols:(ci + 1) * ncols], ph[:], bh_sb[:, 0:1])

            nc.sync.dma_start(out_d, out_sb[:])
    nc.compile()
    return nc


def _make_nc():
    # Bacc (not raw Bass): its compile() runs move_matmul_waits_to_ldweights +
    # generate_event_semaphores, required to satisfy TRN2's 1-wait-per-inst limit.
    return bacc.Bacc("TRN2", target_bir_lowering=False, debug=False,
                     num_devices=NCORES, enable_partition_id=False)


def _to_bf16(a):
    import ml_dtypes
    return a.astype(ml_dtypes.bfloat16)


def _in_maps(inputs):
    folded = _fold_weights(inputs)
    x = np.asarray(inputs["x"], dtype=np.float32)
    maps = []
    for b in range(NCORES):
        m = dict(folded)
        m["xt"] = _to_bf16(np.ascontiguousarray(x[b].reshape(T, CIN, HW)))
        maps.append(m)
    return maps


def _assemble(results):
    out = np.empty((NCORES, H, W), np.float32)
    for b in range(NCORES):
        o = results[b]["out"]          # [128, HW//128], o[i, j] = pixel j*128+i
        out[b] = o.T.reshape(H, W)
    return out


def _run(inputs, trace=False):
    nc = build(_make_nc())
    maps = _in_maps(inputs)
    res = run_bass_kernel_spmd(nc, maps, core_ids=list(range(NCORES)), trace=trace)
    return _assemble(res.results), res


def kernel(**inputs) -> np.ndarray:
    out, _ = _run(inputs, trace=False)
    return out


# revision 12
# speedup vs baseline: 1.4194x; 1.4194x over previous
"""ConvLSTM (pixel-wise, 1x1 convs) Trainium2 Bass kernel.

Math (after exact algebraic folding):
  per pixel, per t:  g1 = W1x @ x_t + W1h @ h1 + b1   (W1x = Wih1 @ (W_red * denorm_scale))
                     i,f,g,o = split(g1); c1 = sig(f)*c1 + sig(i)*tanh(g); h1 = sig(o)*tanh(c1)
                     g2 = W21 @ h1 + W22 @ h2 + b2    (W21 = Wih2 @ Wc1)
                     c2,h2 analogous
  out = (W_head @ Wc2) @ h2_final + const

Sharding: batch b -> core b (8 cores, no collectives).

Per-core layout:
  S1 [92, CHUNK]   rows 0:64 = h1, rows 64:92 = x(t)    (matmul rhs, K=92)
  S2 [128, CHUNK]  rows 0:64 = h1 (dup), 64:128 = h2    (matmul rhs, K=128)
  c1/c2 [128, HALF] : A-half pixels on partitions 0:64, B-half on 64:128
  gate planes [128, FD] in PSUM: per-gate, A-half rows 0:64 / B-half rows 64:128
  -> every ACT/DVE pointwise op runs with all 128 partitions busy.

x is converted to bf16 host-side and DMA'd directly into S1's x rows each
timestep (no staging copy).  All pointwise traffic is bf16 on VectorE
(2x TT mode, 4x copy mode); GPSIMD is not used for compute.
"""

import numpy as np

import concourse.bass as bass
import concourse.tile as tile
from concourse import bacc, mybir
from concourse.bass_utils import run_bass_kernel_spmd

F32 = mybir.dt.float32
BF16 = mybir.dt.bfloat16
AF = mybir.ActivationFunctionType

T, CIN, HID = 8, 28, 64
H = W = 128
HW = H * W            # pixels per core (one batch element)
NCORES = 8
K1, K2 = HID + CIN, 2 * HID   # S1 rows 0:64 = h1, 64:92 = x; S2 rows 0:64 = h1, 64:128 = h2

import os
CFG = dict(
    chunk=8192,        # pixels resident per chunk
    fd=2048,           # PSUM gate-tile free dim (pixels per half per block)
    nt=512,            # matmul moving tile (one PSUM bank of fp32)
    planes_bufs=2,
    pl_dtype="f32",    # gate/chain plane dtype: f32 ACT-writes ~30% faster
    c_dtype="f32",     # cell-state dtype
    copy_mode="dma",   # h-row copies: "dma" (idle DMA engines) | "vector"
)
for _k in list(CFG):
    _v = os.environ.get(f"KCFG_{_k.upper()}")
    if _v is not None:
        CFG[_k] = int(_v) if _v.isdigit() else _v


def _fold_weights(inputs):
    """Host-side exact algebraic folding (all fp32 numpy)."""
    f = np.float32
    W_red = inputs["W_red"].astype(f)
    b_red = inputs["b_red"].astype(f)
    # de-normalization of channels 11 (u) and 12 (v), folded into W_red
    a = np.ones(CIN, f); a[11] = f(0.15); a[12] = f(0.12)
    d = np.zeros(CIN, f); d[11] = f(0.02); d[12] = f(-0.01)
    W_red_eff = W_red * a[None, :]
    b_red_eff = b_red + W_red @ d

    W1x = inputs["Wih1"].astype(f) @ W_red_eff          # [256, 28]
    W1h = inputs["Whh1"].astype(f)                      # [256, 64]
    b1 = (inputs["bih1"] + inputs["bhh1"]).astype(f) + inputs["Wih1"].astype(f) @ b_red_eff
    W21 = inputs["Wih2"].astype(f) @ inputs["Wc1"].astype(f)   # [256, 64]
    W22 = inputs["Whh2"].astype(f)                      # [256, 64]
    b2 = (inputs["bih2"] + inputs["bhh2"]).astype(f) + inputs["Wih2"].astype(f) @ inputs["bc1"].astype(f)
    whead = (inputs["W_head"].astype(f) @ inputs["Wc2"].astype(f))[0]     # [64]
    bhead = float((inputs["W_head"].astype(f) @ inputs["bc2"].astype(f) + inputs["b_head"].astype(f)).reshape(()))

    w1 = np.ascontiguousarray(np.concatenate([W1h, W1x], axis=1).T)  # [92, 256]: h1 rows then x rows
    w2 = np.ascontiguousarray(np.concatenate([W21, W22], axis=1).T)  # [128, 256]
    # per-gate bias vectors duplicated across the two half-planes -> [128, 4]
    bdup = lambda b: np.stack([np.concatenate([b[64 * q:64 * q + 64]] * 2) for q in range(4)], axis=1)
    wh = np.zeros((128, 1), f); wh[64:, 0] = whead
    return dict(w1=w1, w2=w2, b1=np.ascontiguousarray(bdup(b1)),
                b2=np.ascontiguousarray(bdup(b2)), wh=wh,
                bh=np.full((128, 1), bhead, f))


def build(nc):
    chunk = CFG["chunk"]; fd = CFG["fd"]; nt = CFG["nt"]
    nchunk = HW // chunk
    half = chunk // 2
    nblk = half // fd
    nsub = fd // nt
    PL = {"f32": F32, "bf16": BF16}[CFG["pl_dtype"]]
    CD = {"f32": F32, "bf16": BF16}[CFG["c_dtype"]]

    def hcopy(dst, src):
        if CFG["copy_mode"] == "dma":
            nc.sync.dma_start(dst, src)
        else:
            nc.vector.tensor_copy(dst, src)

    x_d = nc.dram_tensor("xt", [T, CIN, HW], BF16, kind="ExternalInput").ap()
    w1_d = nc.dram_tensor("w1", [K1, 256], F32, kind="ExternalInput").ap()
    w2_d = nc.dram_tensor("w2", [K2, 256], F32, kind="ExternalInput").ap()
    wh_d = nc.dram_tensor("wh", [128, 1], F32, kind="ExternalInput").ap()
    b1_d = nc.dram_tensor("b1", [128, 4], F32, kind="ExternalInput").ap()
    b2_d = nc.dram_tensor("b2", [128, 4], F32, kind="ExternalInput").ap()
    bh_d = nc.dram_tensor("bh", [128, 1], F32, kind="ExternalInput").ap()
    # out[i, j] = pixel j*128 + i of this core's [H, W] map (host transposes)
    out_d = nc.dram_tensor("out", [128, HW // 128], F32, kind="ExternalOutput").ap()

    with tile.TileContext(nc) as tc:
        with (
            tc.tile_pool(name="const", bufs=1) as const,
            tc.tile_pool(name="state", bufs=1) as state,
            tc.tile_pool(name="planes", bufs=CFG["planes_bufs"]) as planes,
            tc.tile_pool(name="outp", bufs=1) as outp,
            tc.tile_pool(name="psum", bufs=1, space=bass.MemorySpace.PSUM) as psum,
        ):
            # Stage weights via fp32 tiles + one convert copy each, so every
            # matmul waits on a single compute producer (the fused LDWEIGHTS
            # has very few sync-wait slots; direct multi-queue DMA deps
            # overflow it -> walrus "Too many sync wait commands").
            w1f = const.tile([K1, 256], F32, tag="w1f")
            w2f = const.tile([K2, 256], F32, tag="w2f")
            whf = const.tile([128, 1], F32, tag="whf")
            nc.sync.dma_start(w1f[:], w1_d)
            nc.sync.dma_start(w2f[:], w2_d)
            nc.sync.dma_start(whf[:], wh_d)
            w1_sb = const.tile([K1, 256], BF16, tag="w1")
            w2_sb = const.tile([K2, 256], BF16, tag="w2")
            wh_sb = const.tile([128, 1], BF16, tag="wh")
            nc.vector.tensor_copy(w1_sb[:], w1f[:])
            nc.vector.tensor_copy(w2_sb[:], w2f[:])
            nc.vector.tensor_copy(wh_sb[:], whf[:])
            b1_sb = const.tile([128, 4], F32, tag="b1")
            b2_sb = const.tile([128, 4], F32, tag="b2")
            bh_sb = const.tile([128, 1], F32, tag="bh")
            nc.sync.dma_start(b1_sb[:], b1_d)
            nc.sync.dma_start(b2_sb[:], b2_d)
            nc.sync.dma_start(bh_sb[:], bh_d)

            out_sb = outp.tile([128, HW // 128], F32, tag="osb")

            for ci in range(nchunk):
                px0 = ci * chunk
                S1 = state.tile([K1, chunk], BF16, tag="S1")
                S2 = state.tile([K2, chunk], BF16, tag="S2")
                c1 = state.tile([128, half], CD, tag="c1")
                c2 = state.tile([128, half], CD, tag="c2")

                for t in range(T):
                    # x(t) straight into S1's x rows (host pre-converted bf16).
                    # WAR on last t's L1 matmuls clears early (L1 runs first),
                    # so this overlaps the previous step's L2 phase.
                    nc.sync.dma_start(S1[HID:K1, :], x_d[t][:, px0:px0 + chunk])

                    for lst in (0, 1):
                        if lst == 0:
                            w_sb, b_sb, SS = w1_sb, b1_sb, S1
                            ks = slice(0, K1) if t > 0 else slice(HID, K1)
                            cc = c1
                        else:
                            w_sb, b_sb, SS = w2_sb, b2_sb, S2
                            ks = slice(0, K2) if t > 0 else slice(0, HID)
                            cc = c2

                        # per-block gate planes; ACT fills them from ping-pong
                        # PSUM tiles (2 x [128, fd] f32 = all 8 banks)
                        sis, sfs, tgs, sos = [], [], [], []
                        pp = 0
                        for hb in range(nblk):
                            a0 = hb * fd            # A-half cols in S1/S2
                            b0 = half + hb * fd     # B-half cols
                            blk = slice(hb * fd, (hb + 1) * fd)
                            si = planes.tile([128, fd], PL, tag="si")
                            if t > 0:
                                sf = planes.tile([128, fd], PL, tag="sf")
                            else:
                                sf = None
                            tg = planes.tile([128, fd], PL, tag="tg")
                            so = planes.tile([128, fd], PL, tag="so")
                            sis.append(si); sfs.append(sf); tgs.append(tg); sos.append(so)
                            for (pl, fn, q) in ((si, AF.Sigmoid, 0), (sf, AF.Sigmoid, 1),
                                                (tg, AF.Tanh, 2), (so, AF.Sigmoid, 3)):
                                if pl is None:
                                    continue        # f-gate unused at t=0
                                P = psum.tile([128, fd], F32, tag=f"P{pp % 2}",
                                              name=f"P{pp % 2}")
                                pp += 1
                                for s in range(nsub):
                                    for (cb, po) in ((a0, 0), (b0, 64)):
                                        nc.tensor.matmul(
                                            P[po:po + 64, s * nt:(s + 1) * nt],
                                            w_sb[ks, q * 64:(q + 1) * 64],
                                            SS[ks, cb + s * nt:cb + (s + 1) * nt],
                                        )
                                nc.scalar.activation(pl[:], P[:], fn,
                                                     bias=b_sb[:, q:q + 1])
                            if t > 0:
                                t1 = planes.tile([128, fd], PL, tag="t1")
                                t2 = planes.tile([128, fd], PL, tag="t2")
                                nc.vector.tensor_mul(t1[:], sf[:], cc[:, blk])
                                nc.vector.tensor_mul(t2[:], si[:], tg[:])
                                nc.vector.tensor_add(cc[:, blk], t1[:], t2[:])
                            else:
                                nc.vector.tensor_mul(cc[:, blk], si[:], tg[:])
                        # post-chain per block so the next layer's matmuls can
                        # start as soon as block 0's h-rows land
                        for hb in range(nblk):
                            a0 = hb * fd
                            b0 = half + hb * fd
                            blk = slice(hb * fd, (hb + 1) * fd)
                            so = sos[hb]
                            tch = planes.tile([128, fd], PL, tag="tc")
                            nc.scalar.activation(tch[:], cc[:, blk], AF.Tanh)
                            hp = planes.tile([128, fd], BF16, tag="hp")
                            nc.vector.tensor_mul(hp[:], so[:], tch[:])
                            if lst == 0:
                                # S2's h1 rows feed this step's L2 matmuls: first.
                                hcopy(S2[0:HID, a0:a0 + fd], hp[0:64, :])
                                hcopy(S2[0:HID, b0:b0 + fd], hp[64:128, :])
                                if t < T - 1:
                                    # S1's h1 rows are only needed at t+1.
                                    hcopy(S1[0:HID, a0:a0 + fd], hp[0:64, :])
                                    hcopy(S1[0:HID, b0:b0 + fd], hp[64:128, :])
                            else:
                                hcopy(S2[HID:K2, a0:a0 + fd], hp[0:64, :])
                                hcopy(S2[HID:K2, b0:b0 + fd], hp[64:128, :])

                # head: out[pix] = whead @ h2[pix] + bh, pixels as matmul M-dim
                ncols = chunk // 128
                ph = psum.tile([128, ncols], F32, tag="P0", name="ph")
                for j in range(ncols):
                    nc.tensor.matmul(
                        ph[:, j:j + 1],
                        S2[HID:K2, j * 128:(j + 1) * 128],
                        wh_sb[64:128, 0:1],
                    )
                nc.vector.tensor_scalar_add(
                    out_sb[:, ci * ncols:(ci + 1) * ncols], ph[:], bh_sb[:, 0:1])

            nc.sync.dma_start(out_d, out_sb[:])
    nc.compile()
    return nc


def _make_nc():
    # Bacc (not raw Bass): its compile() runs move_matmul_waits_to_ldweights +
    # generate_event_semaphores, required to satisfy TRN2's 1-wait-per-inst limit.
    return bacc.Bacc("TRN2", target_bir_lowering=False, debug=False,
                     num_devices=NCORES, enable_partition_id=False)


def _to_bf16(a):
    import ml_dtypes
    return a.astype(ml_dtypes.bfloat16)


def _in_maps(inputs):
    folded = _fold_weights(inputs)
    x = np.asarray(inputs["x"], dtype=np.float32)
    maps = []
    for b in range(NCORES):
        m = dict(folded)
        m["xt"] = _to_bf16(np.ascontiguousarray(x[b].reshape(T, CIN, HW)))
        maps.append(m)
    return maps


def _assemble(results):
    out = np.empty((NCORES, H, W), np.float32)
    for b in range(NCORES):
        o = results[b]["out"]          # [128, HW//128], o[i, j] = pixel j*128+i
        out[b] = o.T.reshape(H, W)
    return out


def _run(inputs, trace=False):
    nc = build(_make_nc())
    maps = _in_maps(inputs)
    res = run_bass_kernel_spmd(nc, maps, core_ids=list(range(NCORES)), trace=trace)
    return _assemble(res.results), res


def kernel(**inputs) -> np.ndarray:
    out, _ = _run(inputs, trace=False)
    return out
